# revision 21
# baseline (speedup 1.0000x reference)
"""Trainium2 Bass kernel for nn_CrossAttentionFusion.

Math: softmax over kv_len==1 is identically 1.0, so the attention output is
v broadcast over the N (patch) axis and the whole module reduces to

    out[b, n, :] = cnn[b] @ (Wkv[:, C:] @ Wp) + bp        (independent of n)

W_eff = Wkv[:, C:] @ Wp is a weight-only constant, folded on the host.

Sharding: 8 cores = 4 batch-groups x 2 column-groups. Each core computes
y = cnn_shard @ W_eff_slice + bp_slice for its 16 batches x 384 columns and
writes a [16, 576, 384] output block (14.16 MB; the kernel is bound by this
HBM write stream at ~417 GB/s).

Measured machine behavior this schedule is built around:
  * a transfer's completion sem fires ~2us after its last byte (HBM
    receipt latency), and per-queue throughput scales with descriptor
    size (~150 GB/s at 1536B up to ~210+ GB/s at 6144B per queue);
  * tiny transfers at a ring head cost ~2-3us of ring latency, so the
    bias rides as a 512-col tail on a cnn half (partition 0) instead of
    its own transfer, and is accumulated LAST (K=1 chunk, bf16);
  * the PE HAM (di/dt) throttle runs matmuls at 320ns pitch (vs 162ns
    warm) and RE-ENGAGES after idle gaps, and every wait costs ~1us of
    LDW pipeline refill - so the 16 K-chunk matmuls must consume weight
    groups no faster than the sems arrive: 8 groups of 2 k-chunks
    alternate rings, giving ~1us sem cadence vs 0.64us throttled
    consumption per pair;
  * everything streams in bf16 (error ~2e-3 << 2e-2 gate); the cnn shard
    is host-replicated 8x along the lhsT M axis so the accumulation
    produces y replicated across all 128 PSUM partitions directly;
  * the replicated row is materialized 4x in SBUF (copies alternate
    Vector/Scalar engines to halve the chain) so the 9 write DMAs carry
    6144B descriptors (417 GB/s, and keeps slow SDMA engine 15 in step);
    the first two writes source from the bc4 prefix to start earlier.
"""

import sys

sys.path.insert(0, "/opt/trn_rl_repo")

import ml_dtypes
import numpy as np

import concourse.bass as bass
import concourse.mybir as mybir
from concourse import bacc
from concourse.bass_utils import run_bass_kernel_spmd
from concourse.tile import TileContext

F32 = mybir.dt.float32
BF16 = mybir.dt.bfloat16
NPBF16 = np.dtype(ml_dtypes.bfloat16)

NCORES = 8
B, N, C, CNN = 64, 576, 768, 2048
BGROUPS, CGROUPS = 4, 2          # batch groups x column groups
BS = B // BGROUPS                # 16 batches per core
CW = C // CGROUPS                # 384 columns per core
KC = CNN // 128                  # 16 k-chunks
REP = 128 // BS                  # 8 partitions per batch
ROWS_PP = N // REP               # 72 output rows per partition
RPT = 8                          # rows per partition per write DMA
NWR = ROWS_PP // RPT             # 9 write DMAs
WG = 2                           # weight k-chunks per group transfer
CTAIL = 512                      # bias cols appended to cnn_h1: ones|bp
NCOPIES = 4                      # replicated row copies in SBUF (desc size)


def _build_bass():
    nc = bacc.Bacc(None, target_bir_lowering=False, debug=False, num_devices=NCORES)

    x_cnn = nc.declare_dram_parameter(
        "cnnrep", [128, KC * 128 + CTAIL], BF16, isOutput=False
    )
    x_weff = nc.declare_dram_parameter("weff", [128, KC * CW], BF16, isOutput=False)
    y = nc.declare_dram_parameter("out", [BS, N, CW], F32, isOutput=True)

    with TileContext(nc) as tc:
        with (
            tc.tile_pool(name="singles", bufs=1) as singles,
            tc.tile_pool(name="psum_y", bufs=1, space="PSUM") as psum_y,
        ):
            # PE warm-up: junk fp32 matmul (2 passes, ~2us busy) ramps the
            # HAM di/dt throttle while the first reads stream.
            wu_sb = singles.tile([128, 512], F32, tag="wu_sb")
            nc.gpsimd.memset(wu_sb[:], 0.0)
            with tc.tile_pool(name="psum_w", bufs=1, space="PSUM") as psum_w:
                ps_w = psum_w.tile([8, 512], F32, tag="ps_w")
                nc.tensor.matmul(
                    ps_w[:], wu_sb[:, 0:8], wu_sb[:, :], start=True, stop=True
                )
                ps_w2 = psum_w.tile([8, 384], F32, tag="ps_w2")
                nc.tensor.matmul(
                    ps_w2[:], wu_sb[:, 0:8], wu_sb[:, 0:384], start=True, stop=True
                )

            # --- read streams: cnn halves first, then 2-chunk weight
            # groups alternating rings (staggered sems, rate-matched to
            # throttled matmul consumption) ----------------------------
            half = KC * 128 // 2
            cnn_a = singles.tile([128, half], BF16, tag="cnn_a")
            cnn_b = singles.tile([128, half + CTAIL], BF16, tag="cnn_b")

            def cnn_chunk(kc):
                t = cnn_a if kc < KC // 2 else cnn_b
                o = kc % (KC // 2)
                return t[:, o * 128 : (o + 1) * 128]

            wtiles = [
                singles.tile([128, WG * CW], BF16, tag=f"wg{g}", name=f"wg{g}")
                for g in range(KC // WG)
            ]
            # wg0 heads the scalar ring so kc0's gate (max of cnn_a, wg0
            # sems) is the cnn half, not a weight group queued behind it.
            def wdma(eng, g):
                eng.dma_start(
                    out=wtiles[g][:],
                    in_=x_weff[:, g * WG * CW : (g + 1) * WG * CW],
                )

            nc.sync.dma_start(out=cnn_a[:], in_=x_cnn[:, 0:half])
            wdma(nc.scalar, 0)
            wdma(nc.sync, 1)
            nc.scalar.dma_start(out=cnn_b[:], in_=x_cnn[:, half:])
            wdma(nc.sync, 3)
            wdma(nc.scalar, 2)
            wdma(nc.sync, 5)
            wdma(nc.scalar, 4)
            wdma(nc.sync, 7)
            wdma(nc.scalar, 6)

            # --- compute: y replicated across 128 partitions -----------
            ps_y = psum_y.tile([128, CW], F32, tag="ps_y")
            for kc in range(KC):
                nc.tensor.matmul(
                    ps_y[:],
                    cnn_chunk(kc),
                    wtiles[kc // WG][:, (kc % WG) * CW : (kc % WG + 1) * CW],
                    start=(kc == 0),
                    stop=False,
                )
            # bias last: ps_y[p, c] += 1 * bp[c]; ones|bp ride cnn_b's
            # tail on partition 0
            nc.tensor.matmul(
                ps_y[:],
                cnn_b[0:1, half : half + 128],
                cnn_b[0:1, half + 128 : half + 128 + CW],
                start=False,
                stop=True,
            )

            # materialize NCOPIES of the row; alternate Vector/Scalar
            # engines so the copy chain is ~2x shorter
            bc4 = singles.tile([128, NCOPIES * CW], F32, tag="bc4")
            for j in range(NCOPIES):
                if j % 2 == 0:
                    nc.vector.tensor_copy(bc4[:, j * CW : (j + 1) * CW], ps_y[:])
                else:
                    nc.scalar.copy(bc4[:, j * CW : (j + 1) * CW], ps_y[:])

            # out rows n = q*72 + s for partition p = b*8 + q; each DMA
            # writes RPT consecutive rows per partition. Sources grow with
            # the bc4 prefix so early writes launch before all copies land.
            y_v = y.rearrange("b (q s) c -> (b q) s c", q=REP)
            srcs = {
                0: bc4[:, 0:CW].unsqueeze(1).broadcast_to((128, RPT, CW)),
                1: bc4[:, 0 : 2 * CW]
                .unsqueeze(1)
                .broadcast_to((128, RPT // 2, 2 * CW)),
            }
            src_full = (
                bc4[:, :]
                .unsqueeze(1)
                .broadcast_to((128, RPT // NCOPIES, NCOPIES * CW))
            )
            for i in range(NWR):
                eng = nc.sync if i % 2 == 0 else nc.scalar
                eng.dma_start(
                    out=y_v[:, i * RPT : (i + 1) * RPT, :],
                    in_=srcs.get(i, src_full),
                )

    nc.compile()
    return nc


_NC = None


def _get_nc():
    global _NC
    if _NC is None:
        _NC = _build_bass()
    return _NC


def _prepare_in_maps(image_patches, cnn_feature_vector, Wq, Wkv, Wp, bp):
    Weff = np.ascontiguousarray(Wkv[:, C:]) @ Wp  # (2048, 768) fp32
    bp = bp.astype(np.float32)

    weff_arrs = []
    for cg in range(CGROUPS):
        sl = slice(cg * CW, (cg + 1) * CW)
        weff_arrs.append(
            np.ascontiguousarray(
                Weff[:, sl]
                .reshape(KC, 128, CW)
                .transpose(1, 0, 2)
                .reshape(128, KC * CW)
                .astype(NPBF16)
            )
        )

    cnn_arrs = []
    for bg in range(BGROUPS):
        shard = cnn_feature_vector[bg * BS : (bg + 1) * BS]  # (16, 2048)
        rep = np.repeat(shard, REP, axis=0)  # (128, 2048), row p = batch p//8
        arr = np.zeros((128, KC * 128 + CTAIL), dtype=NPBF16)
        arr[:, : KC * 128] = (
            rep.reshape(128, KC, 128).transpose(2, 1, 0).reshape(128, KC * 128)
        ).astype(NPBF16)
        # bias tail on partition 0: ones(128) | bp-slice placeholder
        arr[0, KC * 128 : KC * 128 + 128] = np.float32(1.0)
        cnn_arrs.append(arr)

    in_maps = []
    for core in range(NCORES):
        bg, cg = core // CGROUPS, core % CGROUPS
        arr = cnn_arrs[bg].copy()
        arr[0, KC * 128 + 128 : KC * 128 + 128 + CW] = bp[
            cg * CW : (cg + 1) * CW
        ].astype(NPBF16)
        in_maps.append({"cnnrep": arr, "weff": weff_arrs[cg]})
    return in_maps


def _assemble(res):
    out = np.empty((B, N, C), dtype=np.float32)
    for core in range(NCORES):
        bg, cg = core // CGROUPS, core % CGROUPS
        out[bg * BS : (bg + 1) * BS, :, cg * CW : (cg + 1) * CW] = res.results[
            core
        ]["out"]
    return out


def kernel(**inputs) -> np.ndarray:
    inputs = {k: np.asarray(v) for k, v in inputs.items()}
    nc = _get_nc()
    in_maps = _prepare_in_maps(**inputs)
    res = run_bass_kernel_spmd(nc, in_maps, core_ids=list(range(NCORES)))
    return _assemble(res)


def kernel_traced(**inputs):
    """kernel() + HW profile; returns (output, BassKernelResults)."""
    inputs = {k: np.asarray(v) for k, v in inputs.items()}
    nc = _get_nc()
    in_maps = _prepare_in_maps(**inputs)
    res = run_bass_kernel_spmd(
        nc, in_maps, core_ids=list(range(NCORES)), trace=True
    )
    return _assemble(res), res


# revision 22
# speedup vs baseline: 1.1352x; 1.1352x over previous
"""Trainium2 Bass kernel for nn_CrossAttentionFusion.

Math: softmax over kv_len==1 is identically 1.0, so the attention output is
v broadcast over the N (patch) axis and the whole module reduces to

    out[b, n, :] = cnn[b] @ (Wkv[:, C:] @ Wp) + bp        (independent of n)

W_eff = Wkv[:, C:] @ Wp is a weight-only constant, folded on the host.

Sharding: 8 cores = 4 batch-groups x 2 column-groups. Each core computes
y = cnn_shard @ W_eff_slice + bp_slice for its 16 batches x 384 columns and
writes [16, 576, 384] (14.16 MB; the kernel is HBM-write bound).

The 384 columns are processed as two halves A/B with separate DRAM outputs
so the A write stream launches while B weights are still streaming - the
DMA rings never idle between the read and write phases. Measured behavior
this schedule is built around:
  * a transfer's completion sem fires ~2us after its last byte, so A's
    weights stream in four 196KB groups whose sems stagger at the
    throttled matmul consumption rate; B's stream in two fat 393KB
    groups behind them;
  * tiny transfers poison a ring head (~2-3us each): the bias rides as a
    512-col tail on cnn_b (partition 0) and is accumulated as a K=1
    bf16 chunk at the end of each half's accumulation;
  * the PE HAM (di/dt) throttle runs matmuls at 320ns pitch (162ns warm)
    and re-engages after idle, so a junk fp32 warm-up matmul precedes;
  * everything streams in bf16 (error ~2e-3 << 2e-2 gate); the cnn shard
    is host-replicated 8x along the lhsT M axis so accumulation yields y
    replicated across all 128 PSUM partitions (partition p = batch p//8);
  * per half, the row is materialized 4x in SBUF (Vector+Scalar engines
    in parallel) giving 3072B write descriptors; the first writes source
    from the prefix to start earlier.
"""

import sys

sys.path.insert(0, "/opt/trn_rl_repo")

import ml_dtypes
import numpy as np

import concourse.bass as bass
import concourse.mybir as mybir
from concourse import bacc
from concourse.bass_utils import run_bass_kernel_spmd
from concourse.tile import TileContext

F32 = mybir.dt.float32
BF16 = mybir.dt.bfloat16
NPBF16 = np.dtype(ml_dtypes.bfloat16)

NCORES = 8
B, N, C, CNN = 64, 576, 768, 2048
BGROUPS, CGROUPS = 4, 2          # batch groups x column groups
BS = B // BGROUPS                # 16 batches per core
CW = C // CGROUPS                # 384 columns per core
HW = CW // 2                     # 192 columns per half
KC = CNN // 128                  # 16 k-chunks
REP = 128 // BS                  # 8 partitions per batch
ROWS_PP = N // REP               # 72 output rows per partition
RPT = 8                          # rows per partition per write DMA
NWR = ROWS_PP // RPT             # 9 write DMAs per half
CTAIL = 512                      # bias cols appended to cnn_b: ones|bp
NCOPIES = 4                      # replicated row copies per half
AG = 4                           # A-half k-chunks per group (4 groups)
BG = 8                           # B-half k-chunks per group (2 groups)


def _build_bass():
    nc = bacc.Bacc(None, target_bir_lowering=False, debug=False, num_devices=NCORES)

    x_cnn = nc.declare_dram_parameter(
        "cnnrep", [128, KC * 128 + CTAIL], BF16, isOutput=False
    )
    # A-half chunks first (kc-major), then B-half
    x_weff = nc.declare_dram_parameter(
        "weff", [128, 2 * KC * HW], BF16, isOutput=False
    )
    ya = nc.declare_dram_parameter("outA", [BS, N, HW], F32, isOutput=True)
    yb = nc.declare_dram_parameter("outB", [BS, N, HW], F32, isOutput=True)

    with TileContext(nc) as tc:
        with (
            tc.tile_pool(name="singles", bufs=1) as singles,
            tc.tile_pool(name="psum_y", bufs=1, space="PSUM") as psum_y,
        ):
            # PE warm-up: junk fp32 matmul ramps the HAM di/dt throttle.
            wu_sb = singles.tile([128, 512], F32, tag="wu_sb")
            nc.gpsimd.memset(wu_sb[:], 0.0)
            with tc.tile_pool(name="psum_w", bufs=1, space="PSUM") as psum_w:
                ps_w = psum_w.tile([8, 512], F32, tag="ps_w")
                nc.tensor.matmul(
                    ps_w[:], wu_sb[:, 0:8], wu_sb[:, :], start=True, stop=True
                )

            # --- read streams -------------------------------------------
            half = KC * 128 // 2
            cnn_a = singles.tile([128, half], BF16, tag="cnn_a")
            cnn_b = singles.tile([128, half + CTAIL], BF16, tag="cnn_b")

            def cnn_chunk(kc):
                t = cnn_a if kc < KC // 2 else cnn_b
                o = kc % (KC // 2)
                return t[:, o * 128 : (o + 1) * 128]

            wa = [
                singles.tile([128, AG * HW], BF16, tag=f"wa{g}", name=f"wa{g}")
                for g in range(KC // AG)
            ]
            wb = [
                singles.tile([128, BG * HW], BF16, tag=f"wb{g}", name=f"wb{g}")
                for g in range(KC // BG)
            ]

            def wa_dma(eng, g):
                eng.dma_start(
                    out=wa[g][:], in_=x_weff[:, g * AG * HW : (g + 1) * AG * HW]
                )

            def wb_dma(eng, g):
                lo = KC * HW + g * BG * HW
                eng.dma_start(out=wb[g][:], in_=x_weff[:, lo : lo + BG * HW])

            # sync: cnn_a, A0, A2, B0; scalar: A1, cnn_b, A3, B1
            nc.sync.dma_start(out=cnn_a[:], in_=x_cnn[:, 0:half])
            wa_dma(nc.scalar, 1)
            wa_dma(nc.sync, 0)
            nc.scalar.dma_start(out=cnn_b[:], in_=x_cnn[:, half:])
            wa_dma(nc.sync, 2)
            wa_dma(nc.scalar, 3)
            wb_dma(nc.sync, 0)
            wb_dma(nc.scalar, 1)

            # --- per-half compute + writes ------------------------------
            def do_half(hname, y, wtiles, gk, bias_lo, worder):
                ps = psum_y.tile([128, HW], F32, tag=f"ps{hname}", name=f"ps{hname}")
                first = True
                for g in worder:
                    for i in range(gk):
                        kc = g * gk + i
                        nc.tensor.matmul(
                            ps[:],
                            cnn_chunk(kc),
                            wtiles[g][:, i * HW : (i + 1) * HW],
                            start=first,
                            stop=False,
                        )
                        first = False
                # bias: ones|bp ride cnn_b's tail on partition 0
                nc.tensor.matmul(
                    ps[:],
                    cnn_b[0:1, half : half + 128],
                    cnn_b[0:1, bias_lo : bias_lo + HW],
                    start=False,
                    stop=True,
                )
                bc4 = singles.tile(
                    [128, NCOPIES * HW], F32, tag=f"bc4{hname}", name=f"bc4{hname}"
                )
                for j in range(NCOPIES):
                    if j % 2 == 0:
                        nc.vector.tensor_copy(bc4[:, j * HW : (j + 1) * HW], ps[:])
                    else:
                        nc.scalar.copy(bc4[:, j * HW : (j + 1) * HW], ps[:])

                y_v = y.rearrange("b (q s) c -> (b q) s c", q=REP)
                srcs = {
                    0: bc4[:, 0:HW].unsqueeze(1).broadcast_to((128, RPT, HW)),
                    1: bc4[:, 0 : 2 * HW]
                    .unsqueeze(1)
                    .broadcast_to((128, RPT // 2, 2 * HW)),
                }
                src_full = (
                    bc4[:, :]
                    .unsqueeze(1)
                    .broadcast_to((128, RPT // NCOPIES, NCOPIES * HW))
                )
                first_eng = nc.sync if hname == "A" else nc.scalar
                second_eng = nc.scalar if hname == "A" else nc.sync
                for i in range(NWR):
                    eng = first_eng if i % 2 == 0 else second_eng
                    eng.dma_start(
                        out=y_v[:, i * RPT : (i + 1) * RPT, :],
                        in_=srcs.get(i, src_full),
                    )

            # A: consume scalar-head group first (its sem fires first)
            do_half("A", ya, wa, AG, half + 128, worder=(1, 0, 2, 3))
            do_half("B", yb, wb, BG, half + 128 + HW, worder=(0, 1))

    nc.compile()
    return nc


_NC = None


def _get_nc():
    global _NC
    if _NC is None:
        _NC = _build_bass()
    return _NC


def _wlayout(w):
    # (2048, HW) -> [128, KC*HW], chunk kc at cols [kc*HW:(kc+1)*HW]
    return (
        w.reshape(KC, 128, HW).transpose(1, 0, 2).reshape(128, KC * HW)
    ).astype(NPBF16)


def _prepare_in_maps(image_patches, cnn_feature_vector, Wq, Wkv, Wp, bp):
    Weff = np.ascontiguousarray(Wkv[:, C:]) @ Wp  # (2048, 768) fp32
    bp = bp.astype(np.float32)

    weff_arrs = []
    for cg in range(CGROUPS):
        lo = cg * CW
        arr = np.empty((128, 2 * KC * HW), dtype=NPBF16)
        arr[:, : KC * HW] = _wlayout(Weff[:, lo : lo + HW])
        arr[:, KC * HW :] = _wlayout(Weff[:, lo + HW : lo + CW])
        weff_arrs.append(arr)

    cnn_arrs = []
    for bg in range(BGROUPS):
        shard = cnn_feature_vector[bg * BS : (bg + 1) * BS]  # (16, 2048)
        rep = np.repeat(shard, REP, axis=0)  # (128, 2048), row p = batch p//8
        arr = np.zeros((128, KC * 128 + CTAIL), dtype=NPBF16)
        arr[:, : KC * 128] = (
            rep.reshape(128, KC, 128).transpose(2, 1, 0).reshape(128, KC * 128)
        ).astype(NPBF16)
        arr[0, KC * 128 : KC * 128 + 128] = np.float32(1.0)
        cnn_arrs.append(arr)

    in_maps = []
    for core in range(NCORES):
        bg, cg = core // CGROUPS, core % CGROUPS
        arr = cnn_arrs[bg].copy()
        arr[0, KC * 128 + 128 : KC * 128 + 128 + CW] = bp[
            cg * CW : (cg + 1) * CW
        ].astype(NPBF16)
        in_maps.append({"cnnrep": arr, "weff": weff_arrs[cg]})
    return in_maps


def _assemble(res):
    out = np.empty((B, N, C), dtype=np.float32)
    for core in range(NCORES):
        bg, cg = core // CGROUPS, core % CGROUPS
        bsl = slice(bg * BS, (bg + 1) * BS)
        lo = cg * CW
        out[bsl, :, lo : lo + HW] = res.results[core]["outA"]
        out[bsl, :, lo + HW : lo + CW] = res.results[core]["outB"]
    return out


def kernel(**inputs) -> np.ndarray:
    inputs = {k: np.asarray(v) for k, v in inputs.items()}
    nc = _get_nc()
    in_maps = _prepare_in_maps(**inputs)
    res = run_bass_kernel_spmd(nc, in_maps, core_ids=list(range(NCORES)))
    return _assemble(res)


def kernel_traced(**inputs):
    """kernel() + HW profile; returns (output, BassKernelResults)."""
    inputs = {k: np.asarray(v) for k, v in inputs.items()}
    nc = _get_nc()
    in_maps = _prepare_in_maps(**inputs)
    res = run_bass_kernel_spmd(
        nc, in_maps, core_ids=list(range(NCORES)), trace=True
    )
    return _assemble(res), res
